# revision 15
# baseline (speedup 1.0000x reference)
"""Trainium2 Bass kernel for nn_Custom_loss_66829691125920.

Computes a CLIP-style loss: symmetric InfoNCE over max-pooled token
similarities (two image-view sets) plus a triplet margin term, on 8
NeuronCores.

Strategy
--------
- Shard the batch dim N=96 across 8 cores (12 rows each, data parallel on v).
- On the host, fold mask + 1/valid into the text tokens (max/sum commute with
  the nonneg per-token scaling), drop masked tokens, and pack the surviving
  tokens of t_pos (shared by all cores) plus each core's own t_neg tokens into
  one padded token stream of Tp = 128*Tb tokens (fp16).
- Per core, per 128-token block: PE matmuls tokens x v -> PSUM sim tiles
  [128 tok, 2*196]; VectorE max-reduces over the 196 image tokens -> word
  scores; a second PE matmul against a 0/1 segment matrix accumulates the
  per-(i, j) pooled similarities S into one PSUM tile [108, 24]
  (segments: 96 pos rows j + 12 own neg rows).
- AllGather the per-core S blocks; every core redundantly computes the final
  scalar (row/col logsumexp, diagonal, triplet relu means) on-device.
"""

import math

import numpy as np

N, P, L, D = 96, 196, 64, 128
NCORES = 8
NL = N // NCORES  # 12 rows per core
MARGIN = 0.7
CLAMP_MAX = 4.6052

_CACHE = {}


def _build_program(Tp, s, dbg=False):
    import concourse.bass as bass
    import concourse.mybir as mybir
    import concourse.tile as tile
    from concourse import bacc
    from concourse.masks import make_identity

    f32 = mybir.dt.float32
    f16 = mybir.dt.float16
    Tb = Tp // 128

    nc = bacc.Bacc("TRN2", target_bir_lowering=False, num_devices=NCORES)
    if dbg:
        d_dpay = nc.dram_tensor("dbg_pay", [128, 26], f32, kind="ExternalOutput")
        d_dsum = nc.dram_tensor("dbg_sumt", [128, 8], f32, kind="ExternalOutput")
        d_dneg = nc.dram_tensor("dbg_negd", [96, 2], f32, kind="ExternalOutput")
        d_ddiag = nc.dram_tensor("dbg_diag", [96, 2], f32, kind="ExternalOutput")
        d_dwb = nc.dram_tensor("dbg_wb", [128, 24], f32, kind="ExternalOutput")

    d_vT = nc.dram_tensor("vT", [128, 2, NL, P], f16, kind="ExternalInput")
    d_tokT = nc.dram_tensor("tokT", [128, Tp], f16, kind="ExternalInput")
    d_seg = nc.dram_tensor("seg", [128, Tb, 108], f16, kind="ExternalInput")
    d_maskN = nc.dram_tensor("maskN", [128, 12], f32, kind="ExternalInput")
    d_wvec = nc.dram_tensor("wvec", [1, 8], f32, kind="ExternalInput")
    d_out = nc.dram_tensor("loss", [1, 1], f32, kind="ExternalOutput")

    with tile.TileContext(nc) as tc:
        with (
            tc.tile_pool(name="const", bufs=1) as cpool,
            tc.tile_pool(name="word", bufs=4) as wpool,
            tc.tile_pool(name="fin", bufs=1) as fpool,
            tc.tile_pool(name="psim", bufs=2, space="PSUM") as spool,
            tc.tile_pool(name="psS", bufs=1, space="PSUM") as sppool,
            tc.tile_pool(name="dram", bufs=1, space="DRAM") as dpool,
        ):
            sb_vT = cpool.tile([128, 2, NL, P], f16)
            sb_tokT = cpool.tile([128, Tp], f16)
            sb_seg = cpool.tile([128, Tb, 108], f16)
            sb_maskN = cpool.tile([128, 12], f32)
            sb_wvec = cpool.tile([1, 8], f32)
            nc.sync.dma_start(sb_vT[:, :, :, :], d_vT[:, :, :, :])
            nc.sync.dma_start(sb_tokT[:, :], d_tokT[:, :])
            nc.sync.dma_start(sb_seg[:, :, :], d_seg[:, :, :])
            nc.sync.dma_start(sb_maskN[:, :], d_maskN[:, :])
            nc.sync.dma_start(sb_wvec[:, :], d_wvec[:, :])

            # ---- main loop (vset-major): sim matmuls + max-pool + segment
            # matmul, then per-vset payload + AllGather so the first gather
            # overlaps the second vset's compute.
            # Max stage: ScalarE drains PSUM to SBUF fp16; VectorE then runs
            # a 2x-mode fp16 pairwise max + a short 1x reduce. Every DIRECT_K-th
            # tile reduces straight from PSUM on VectorE to balance engines.
            DIRECT_K = 12
            tix = 0
            g_ds = []
            for vs in range(2):
                psS = sppool.tile([108, 12], f32, tag=f"psS{vs}")
                for b in range(Tb):
                    wb = wpool.tile([128, 12], f16, tag="word")
                    for half in range(2):
                        ps = spool.tile([128, 3, 512], f32, tag="sim")
                        for k in range(3):
                            pr = half * 3 + k
                            nc.tensor.matmul(
                                ps[:, k, 0 : 2 * P],
                                lhsT=sb_tokT[:, b * 128 : (b + 1) * 128],
                                rhs=sb_vT[:, vs, pr * 2 : pr * 2 + 2, :],
                                start=True,
                                stop=True,
                            )
                        wslice = wb[:, half * 6 : half * 6 + 6]
                        psview = ps[:, :, 0 : 2 * P].rearrange(
                            "p a (b c) -> p a b c", c=P
                        )
                        tix += 1
                        if tix % DIRECT_K == 0:
                            nc.vector.tensor_reduce(
                                out=wslice,
                                in_=psview,
                                axis=mybir.AxisListType.X,
                                op=mybir.AluOpType.max,
                            )
                        else:
                            hh = wpool.tile([128, 3, 2, P], f16, tag="hcopy")
                            nc.scalar.copy(hh[:, :, :, :], psview)
                            cc = wpool.tile([128, 3, 2, P // 2], f16, tag="cmax")
                            nc.vector.tensor_tensor(
                                out=cc[:, :, :, :],
                                in0=hh[:, :, :, 0 : P // 2],
                                in1=hh[:, :, :, P // 2 : P],
                                op=mybir.AluOpType.max,
                            )
                            nc.vector.tensor_reduce(
                                out=wslice,
                                in_=cc[:, :, :, :],
                                axis=mybir.AxisListType.X,
                                op=mybir.AluOpType.max,
                            )
                    nc.tensor.matmul(
                        psS[:, :],
                        lhsT=sb_seg[:, b, :],
                        rhs=wb[:, :],
                        start=(b == 0),
                        stop=(b == Tb - 1),
                        skip_group_check=True,
                    )

                # payload: S block [108,12] + own neg diag in col 12
                payload = fpool.tile([128, 13], f32, tag=f"payload{vs}")
                nc.vector.memset(payload[:, :], 0.0)
                nc.scalar.copy(payload[0:108, 0:12], psS[:, :])
                ntmp = fpool.tile([128, 12], f32, tag=f"ntmp{vs}")
                nc.vector.tensor_tensor(
                    out=ntmp[96:108, :],
                    in0=psS[96:108, :],
                    in1=sb_maskN[96:108, :],
                    op=mybir.AluOpType.mult,
                )
                nc.vector.tensor_reduce(
                    out=payload[96:108, 12:13],
                    in_=ntmp[96:108, :],
                    axis=mybir.AxisListType.X,
                    op=mybir.AluOpType.add,
                )
                pay_d = dpool.tile([128, 13], f32, tag=f"pay{vs}")
                g_d = dpool.tile([NCORES, 128, 13], f32, tag=f"g{vs}")
                nc.sync.dma_start(pay_d[:, :], payload[:, :])
                nc.gpsimd.collective_compute(
                    "AllGather",
                    mybir.AluOpType.bypass,
                    replica_groups=[list(range(NCORES))],
                    ins=[pay_d.opt()],
                    outs=[g_d.opt()],
                )
                g_ds.append(g_d)

            # ---- final (redundant on all cores) ----
            ident = cpool.tile([128, 128], f32)
            make_identity(nc, ident[:, :])
            sb_ones = cpool.tile([128, 1], f32)
            nc.vector.memset(sb_ones[:, :], 1.0)

            sumt = fpool.tile([128, 8], f32)
            nc.vector.memset(sumt[:, :], 0.0)

            for vs in range(2):
                # S^T [j, i]: G[c, j, vs*12+il]
                smt = fpool.tile([96, 96], f32, tag=f"smt{vs}")
                nc.sync.dma_start(
                    smt[:, :].rearrange("j (c il) -> j c il", c=NCORES),
                    g_ds[vs][:, 0:96, 0:12].rearrange("c j il -> j c il"),
                )
                # neg sims as [i, 1] (dest partition dim can't be split in one
                # AP, so copy per source core)
                negd = fpool.tile([96, 1], f32, tag=f"negd{vs}")
                for c in range(NCORES):
                    nc.sync.dma_start(
                        negd[c * NL : (c + 1) * NL, :],
                        g_ds[vs][c, 96:108, 12:13],
                    )

                # transpose -> S [i, j]
                pt = spool.tile([128, 3, 512], f32, tag="sim")
                nc.tensor.transpose(pt[0:96, 0, 0:96], smt[:, :], ident[0:96, 0:96])
                sm = fpool.tile([96, 96], f32, tag=f"sm{vs}")
                nc.scalar.copy(sm[:, :], pt[0:96, 0, 0:96])

                # diag (raw, unscaled)
                dtmp = fpool.tile([96, 96], f32, tag="dtmp")
                nc.vector.tensor_tensor(
                    out=dtmp[:, :],
                    in0=smt[:, :],
                    in1=ident[0:96, 0:96],
                    op=mybir.AluOpType.mult,
                )
                diag = fpool.tile([96, 1], f32, tag=f"diag{vs}")
                nc.vector.tensor_reduce(
                    out=diag[:, :],
                    in_=dtmp[:, :],
                    axis=mybir.AxisListType.X,
                    op=mybir.AluOpType.add,
                )
                nc.vector.tensor_scalar_mul(sumt[0:96, 4 + vs : 5 + vs], diag[:, :], float(s))

                # triplet: relu(MARGIN - diag + negd)
                t1 = fpool.tile([96, 1], f32, tag="t1")
                nc.vector.tensor_scalar(
                    out=t1[:, :],
                    in0=diag[:, :],
                    scalar1=-1.0,
                    scalar2=float(MARGIN),
                    op0=mybir.AluOpType.mult,
                    op1=mybir.AluOpType.add,
                )
                t2 = fpool.tile([96, 1], f32, tag="t2")
                nc.vector.tensor_tensor(
                    out=t2[:, :], in0=t1[:, :], in1=negd[:, :], op=mybir.AluOpType.add
                )
                nc.vector.tensor_scalar_max(sumt[0:96, 6 + vs : 7 + vs], t2[:, :], 0.0)
                if dbg:
                    nc.sync.dma_start(d_dneg[:, vs : vs + 1], negd[:, :])
                    nc.sync.dma_start(d_ddiag[:, vs : vs + 1], diag[:, :])

                # logsumexp along free dim for both orientations
                for col, mat in ((1 + 2 * vs, smt), (0 + 2 * vs, sm)):
                    rm = fpool.tile([96, 1], f32, tag="rm")
                    nc.vector.tensor_reduce(
                        out=rm[:, :],
                        in_=mat[:, :],
                        axis=mybir.AxisListType.X,
                        op=mybir.AluOpType.max,
                    )
                    brm = fpool.tile([96, 1], f32, tag="brm")
                    nc.vector.tensor_scalar_mul(brm[:, :], rm[:, :], -float(s))
                    etmp = fpool.tile([96, 96], f32, tag="etmp")
                    sume = fpool.tile([96, 1], f32, tag="sume")
                    nc.scalar.activation(
                        etmp[:, :],
                        mat[:, :],
                        mybir.ActivationFunctionType.Exp,
                        bias=brm[:, :],
                        scale=float(s),
                        accum_out=sume[:, :],
                    )
                    lg = fpool.tile([96, 1], f32, tag="lg")
                    nc.scalar.activation(
                        lg[:, :], sume[:, :], mybir.ActivationFunctionType.Ln
                    )
                    nc.vector.scalar_tensor_tensor(
                        out=sumt[0:96, col : col + 1],
                        in0=rm[:, :],
                        scalar=float(s),
                        in1=lg[:, :],
                        op0=mybir.AluOpType.mult,
                        op1=mybir.AluOpType.add,
                    )

            if dbg:
                nc.sync.dma_start(d_dsum[:, :], sumt[:, :])

            # column sums via ones-matmul, then weighted total
            po = spool.tile([128, 3, 512], f32, tag="sim")
            nc.tensor.matmul(
                po[0:1, 0, 0:8], lhsT=sb_ones[:, :], rhs=sumt[:, :], start=True, stop=True
            )
            so = fpool.tile([1, 8], f32, tag="so")
            nc.scalar.copy(so[:, :], po[0:1, 0, 0:8])
            sw = fpool.tile([1, 8], f32, tag="sw")
            nc.vector.tensor_tensor(
                out=sw[:, :], in0=so[:, :], in1=sb_wvec[:, :], op=mybir.AluOpType.mult
            )
            res = fpool.tile([1, 1], f32, tag="res")
            nc.vector.tensor_reduce(
                out=res[:, :],
                in_=sw[:, :],
                axis=mybir.AxisListType.X,
                op=mybir.AluOpType.add,
            )
            nc.sync.dma_start(d_out[:, :], res[:, :])

    nc.compile()
    return nc


def _prepare_inputs(inputs):
    v_main = np.asarray(inputs["v_main"], np.float32)
    v_aug = np.asarray(inputs["v_aug"], np.float32)
    t_pos = np.asarray(inputs["t_pos"], np.float32)
    t_neg = np.asarray(inputs["t_neg"], np.float32)
    m_pos = np.asarray(inputs["m_pos"]).astype(bool)
    m_neg = np.asarray(inputs["m_neg"]).astype(bool)
    ls = float(np.asarray(inputs["logit_scale"], np.float32))
    s = float(np.exp(np.clip(ls, 0.0, CLAMP_MAX)))

    valid_pos = np.maximum(m_pos.sum(1), 1).astype(np.float32)
    valid_neg = np.maximum(m_neg.sum(1), 1).astype(np.float32)
    jj, llp = np.nonzero(m_pos)
    Kpos = len(jj)
    pos_tok = t_pos[jj, llp, :] / valid_pos[jj][:, None]
    nii, nll = np.nonzero(m_neg)
    maxKn = max(int(np.sum((nii // NL) == c)) for c in range(NCORES)) if len(nii) else 0
    Tp = 128 * int(math.ceil((Kpos + maxKn) / 128)) if (Kpos + maxKn) else 128
    Tb = Tp // 128

    maskN = np.zeros((128, 12), np.float32)
    for il in range(12):
        maskN[96 + il, il] = 1.0
    wvec = (np.array([[1, 1, 1, 1, -2, -2, 1, 1]], np.float32) / 192.0)

    in_maps = []
    for c in range(NCORES):
        tok = np.zeros((Tp, D), np.float32)
        seg = np.zeros((Tp, 108), np.float32)
        tok[:Kpos] = pos_tok
        seg[np.arange(Kpos), jj] = 1.0
        sel = (nii // NL) == c
        ii, lln = nii[sel], nll[sel]
        kneg = len(ii)
        tok[Kpos : Kpos + kneg] = t_neg[ii, lln, :] / valid_neg[ii][:, None]
        seg[Kpos + np.arange(kneg), 96 + ii % NL] = 1.0

        rows = slice(c * NL, (c + 1) * NL)
        vv = np.stack([v_main[rows], v_aug[rows]])  # [2,12,196,128]
        in_maps.append(
            {
                "vT": np.ascontiguousarray(np.transpose(vv, (3, 0, 1, 2))).astype(
                    np.float16
                ),
                "tokT": np.ascontiguousarray(tok.T).astype(np.float16),
                "seg": np.ascontiguousarray(
                    np.transpose(seg.reshape(Tb, 128, 108), (1, 0, 2))
                ).astype(np.float16),
                "maskN": maskN,
                "wvec": wvec,
            }
        )
    return in_maps, Tp, s


def kernel(_trace=False, **inputs):
    from concourse.bass_utils import run_bass_kernel_spmd

    in_maps, Tp, s = _prepare_inputs(inputs)

    key = (Tp, round(s, 9))
    nc = _CACHE.get(key)
    if nc is None:
        nc = _build_program(Tp, s)
        _CACHE[key] = nc

    br = None
    for attempt in range(3):
        try:
            br = run_bass_kernel_spmd(
                nc, in_maps, core_ids=list(range(NCORES)), trace=_trace
            )
            break
        except ModuleNotFoundError:
            # no axon NTFF hook in this container -> run untraced
            _trace = False
        except Exception:
            # transient NRT_EXEC_UNIT_UNRECOVERABLE on the axon terminal has
            # been observed between back-to-back loads; a retry recovers it
            if attempt == 2:
                raise
            import time as _time

            _time.sleep(5.0)
    assert br is not None
    if _trace and br.exec_time_ns is not None:
        kernel.last_exec_time_ns = br.exec_time_ns
    loss = br.results[0]["loss"]
    return np.asarray(loss, np.float32).reshape(())


kernel.last_exec_time_ns = None


# revision 16
# speedup vs baseline: 1.0057x; 1.0057x over previous
"""Trainium2 Bass kernel for nn_Custom_loss_66829691125920.

Computes a CLIP-style loss: symmetric InfoNCE over max-pooled token
similarities (two image-view sets) plus a triplet margin term, on 8
NeuronCores.

Strategy
--------
- Shard the batch dim N=96 across 8 cores (12 rows each, data parallel on v).
- On the host, fold mask + 1/valid into the text tokens (max/sum commute with
  the nonneg per-token scaling), drop masked tokens, and pack the surviving
  tokens of t_pos (shared by all cores) plus each core's own t_neg tokens into
  one padded token stream of Tp = 128*Tb tokens (fp16).
- Per core, per 128-token block: PE matmuls tokens x v -> PSUM sim tiles
  [128 tok, 2*196]; VectorE max-reduces over the 196 image tokens -> word
  scores; a second PE matmul against a 0/1 segment matrix accumulates the
  per-(i, j) pooled similarities S into one PSUM tile [108, 24]
  (segments: 96 pos rows j + 12 own neg rows).
- AllGather the per-core S blocks; every core redundantly computes the final
  scalar (row/col logsumexp, diagonal, triplet relu means) on-device.
"""

import math

import numpy as np

N, P, L, D = 96, 196, 64, 128
NCORES = 8
NL = N // NCORES  # 12 rows per core
MARGIN = 0.7
CLAMP_MAX = 4.6052

_CACHE = {}


def _build_program(Tp, s, dbg=False):
    import concourse.bass as bass
    import concourse.mybir as mybir
    import concourse.tile as tile
    from concourse import bacc
    from concourse.masks import make_identity

    f32 = mybir.dt.float32
    f16 = mybir.dt.float16
    Tb = Tp // 128

    nc = bacc.Bacc("TRN2", target_bir_lowering=False, num_devices=NCORES)
    if dbg:
        d_dpay = nc.dram_tensor("dbg_pay", [128, 26], f32, kind="ExternalOutput")
        d_dsum = nc.dram_tensor("dbg_sumt", [128, 8], f32, kind="ExternalOutput")
        d_dneg = nc.dram_tensor("dbg_negd", [96, 2], f32, kind="ExternalOutput")
        d_ddiag = nc.dram_tensor("dbg_diag", [96, 2], f32, kind="ExternalOutput")
        d_dwb = nc.dram_tensor("dbg_wb", [128, 24], f32, kind="ExternalOutput")

    d_vT = nc.dram_tensor("vT", [128, 2, NL, P], f16, kind="ExternalInput")
    d_tokT = nc.dram_tensor("tokT", [128, Tp], f16, kind="ExternalInput")
    d_seg = nc.dram_tensor("seg", [128, Tb, 108], f16, kind="ExternalInput")
    d_maskN = nc.dram_tensor("maskN", [128, 12], f32, kind="ExternalInput")
    d_wvec = nc.dram_tensor("wvec", [1, 8], f32, kind="ExternalInput")
    d_out = nc.dram_tensor("loss", [1, 1], f32, kind="ExternalOutput")

    with tile.TileContext(nc) as tc:
        with (
            tc.tile_pool(name="const", bufs=1) as cpool,
            tc.tile_pool(name="word", bufs=4) as wpool,
            tc.tile_pool(name="fin", bufs=1) as fpool,
            tc.tile_pool(name="psim", bufs=2, space="PSUM") as spool,
            tc.tile_pool(name="psS", bufs=1, space="PSUM") as sppool,
            tc.tile_pool(name="dram", bufs=1, space="DRAM") as dpool,
        ):
            sb_vT = cpool.tile([128, 2, NL, P], f16)
            sb_tokT = cpool.tile([128, Tp], f16)
            sb_seg = cpool.tile([128, Tb, 108], f16)
            sb_maskN = cpool.tile([128, 12], f32)
            sb_wvec = cpool.tile([1, 8], f32)
            nc.sync.dma_start(sb_vT[:, :, :, :], d_vT[:, :, :, :])
            nc.sync.dma_start(sb_tokT[:, :], d_tokT[:, :])
            nc.sync.dma_start(sb_seg[:, :, :], d_seg[:, :, :])
            nc.sync.dma_start(sb_maskN[:, :], d_maskN[:, :])
            nc.sync.dma_start(sb_wvec[:, :], d_wvec[:, :])

            # ---- main loop (vset-major): sim matmuls + max-pool + segment
            # matmul, then per-vset payload + AllGather so the first gather
            # overlaps the second vset's compute.
            # Max stage: ScalarE drains PSUM to SBUF fp16; VectorE then runs
            # a 2x-mode fp16 pairwise max + a short 1x reduce. Every DIRECT_K-th
            # tile reduces straight from PSUM on VectorE to balance engines.
            DIRECT_K = 9
            tix = 0
            g_ds = []
            for vs in range(2):
                psS = sppool.tile([108, 12], f32, tag=f"psS{vs}")
                for b in range(Tb):
                    wb = wpool.tile([128, 12], f16, tag="word")
                    for half in range(2):
                        ps = spool.tile([128, 3, 512], f32, tag="sim")
                        for k in range(3):
                            pr = half * 3 + k
                            nc.tensor.matmul(
                                ps[:, k, 0 : 2 * P],
                                lhsT=sb_tokT[:, b * 128 : (b + 1) * 128],
                                rhs=sb_vT[:, vs, pr * 2 : pr * 2 + 2, :],
                                start=True,
                                stop=True,
                            )
                        wslice = wb[:, half * 6 : half * 6 + 6]
                        psview = ps[:, :, 0 : 2 * P].rearrange(
                            "p a (b c) -> p a b c", c=P
                        )
                        tix += 1
                        if tix % DIRECT_K == 0:
                            nc.vector.tensor_reduce(
                                out=wslice,
                                in_=psview,
                                axis=mybir.AxisListType.X,
                                op=mybir.AluOpType.max,
                            )
                        else:
                            hh = wpool.tile([128, 3, 2, P], f16, tag="hcopy")
                            nc.scalar.copy(hh[:, :, :, :], psview)
                            cc = wpool.tile([128, 3, 2, P // 2], f16, tag="cmax")
                            nc.vector.tensor_tensor(
                                out=cc[:, :, :, :],
                                in0=hh[:, :, :, 0 : P // 2],
                                in1=hh[:, :, :, P // 2 : P],
                                op=mybir.AluOpType.max,
                            )
                            nc.vector.tensor_reduce(
                                out=wslice,
                                in_=cc[:, :, :, :],
                                axis=mybir.AxisListType.X,
                                op=mybir.AluOpType.max,
                            )
                    nc.tensor.matmul(
                        psS[:, :],
                        lhsT=sb_seg[:, b, :],
                        rhs=wb[:, :],
                        start=(b == 0),
                        stop=(b == Tb - 1),
                        skip_group_check=True,
                    )

                # payload: S block [108,12] + own neg diag in col 12
                payload = fpool.tile([128, 13], f32, tag=f"payload{vs}")
                nc.vector.memset(payload[:, :], 0.0)
                nc.scalar.copy(payload[0:108, 0:12], psS[:, :])
                ntmp = fpool.tile([128, 12], f32, tag=f"ntmp{vs}")
                nc.vector.tensor_tensor(
                    out=ntmp[96:108, :],
                    in0=psS[96:108, :],
                    in1=sb_maskN[96:108, :],
                    op=mybir.AluOpType.mult,
                )
                nc.vector.tensor_reduce(
                    out=payload[96:108, 12:13],
                    in_=ntmp[96:108, :],
                    axis=mybir.AxisListType.X,
                    op=mybir.AluOpType.add,
                )
                pay_d = dpool.tile([128, 13], f32, tag=f"pay{vs}")
                g_d = dpool.tile([NCORES, 128, 13], f32, tag=f"g{vs}")
                nc.sync.dma_start(pay_d[:, :], payload[:, :])
                nc.gpsimd.collective_compute(
                    "AllGather",
                    mybir.AluOpType.bypass,
                    replica_groups=[list(range(NCORES))],
                    ins=[pay_d.opt()],
                    outs=[g_d.opt()],
                )
                g_ds.append(g_d)

            # ---- final (redundant on all cores) ----
            ident = cpool.tile([128, 128], f32)
            make_identity(nc, ident[:, :])
            sb_ones = cpool.tile([128, 1], f32)
            nc.vector.memset(sb_ones[:, :], 1.0)

            sumt = fpool.tile([128, 8], f32)
            nc.vector.memset(sumt[:, :], 0.0)

            for vs in range(2):
                # S^T [j, i]: G[c, j, vs*12+il]
                smt = fpool.tile([96, 96], f32, tag=f"smt{vs}")
                nc.sync.dma_start(
                    smt[:, :].rearrange("j (c il) -> j c il", c=NCORES),
                    g_ds[vs][:, 0:96, 0:12].rearrange("c j il -> j c il"),
                )
                # neg sims as [i, 1] (dest partition dim can't be split in one
                # AP, so copy per source core)
                negd = fpool.tile([96, 1], f32, tag=f"negd{vs}")
                for c in range(NCORES):
                    nc.sync.dma_start(
                        negd[c * NL : (c + 1) * NL, :],
                        g_ds[vs][c, 96:108, 12:13],
                    )

                # transpose -> S [i, j]
                pt = spool.tile([128, 3, 512], f32, tag="sim")
                nc.tensor.transpose(pt[0:96, 0, 0:96], smt[:, :], ident[0:96, 0:96])
                sm = fpool.tile([96, 96], f32, tag=f"sm{vs}")
                nc.scalar.copy(sm[:, :], pt[0:96, 0, 0:96])

                # diag (raw, unscaled)
                dtmp = fpool.tile([96, 96], f32, tag="dtmp")
                nc.vector.tensor_tensor(
                    out=dtmp[:, :],
                    in0=smt[:, :],
                    in1=ident[0:96, 0:96],
                    op=mybir.AluOpType.mult,
                )
                diag = fpool.tile([96, 1], f32, tag=f"diag{vs}")
                nc.vector.tensor_reduce(
                    out=diag[:, :],
                    in_=dtmp[:, :],
                    axis=mybir.AxisListType.X,
                    op=mybir.AluOpType.add,
                )
                nc.vector.tensor_scalar_mul(sumt[0:96, 4 + vs : 5 + vs], diag[:, :], float(s))

                # triplet: relu(MARGIN - diag + negd)
                t1 = fpool.tile([96, 1], f32, tag="t1")
                nc.vector.tensor_scalar(
                    out=t1[:, :],
                    in0=diag[:, :],
                    scalar1=-1.0,
                    scalar2=float(MARGIN),
                    op0=mybir.AluOpType.mult,
                    op1=mybir.AluOpType.add,
                )
                t2 = fpool.tile([96, 1], f32, tag="t2")
                nc.vector.tensor_tensor(
                    out=t2[:, :], in0=t1[:, :], in1=negd[:, :], op=mybir.AluOpType.add
                )
                nc.vector.tensor_scalar_max(sumt[0:96, 6 + vs : 7 + vs], t2[:, :], 0.0)
                if dbg:
                    nc.sync.dma_start(d_dneg[:, vs : vs + 1], negd[:, :])
                    nc.sync.dma_start(d_ddiag[:, vs : vs + 1], diag[:, :])

                # logsumexp along free dim for both orientations
                for col, mat in ((1 + 2 * vs, smt), (0 + 2 * vs, sm)):
                    rm = fpool.tile([96, 1], f32, tag="rm")
                    nc.vector.tensor_reduce(
                        out=rm[:, :],
                        in_=mat[:, :],
                        axis=mybir.AxisListType.X,
                        op=mybir.AluOpType.max,
                    )
                    brm = fpool.tile([96, 1], f32, tag="brm")
                    nc.vector.tensor_scalar_mul(brm[:, :], rm[:, :], -float(s))
                    etmp = fpool.tile([96, 96], f32, tag="etmp")
                    sume = fpool.tile([96, 1], f32, tag="sume")
                    nc.scalar.activation(
                        etmp[:, :],
                        mat[:, :],
                        mybir.ActivationFunctionType.Exp,
                        bias=brm[:, :],
                        scale=float(s),
                        accum_out=sume[:, :],
                    )
                    lg = fpool.tile([96, 1], f32, tag="lg")
                    nc.scalar.activation(
                        lg[:, :], sume[:, :], mybir.ActivationFunctionType.Ln
                    )
                    nc.vector.scalar_tensor_tensor(
                        out=sumt[0:96, col : col + 1],
                        in0=rm[:, :],
                        scalar=float(s),
                        in1=lg[:, :],
                        op0=mybir.AluOpType.mult,
                        op1=mybir.AluOpType.add,
                    )

            if dbg:
                nc.sync.dma_start(d_dsum[:, :], sumt[:, :])

            # column sums via ones-matmul, then weighted total
            po = spool.tile([128, 3, 512], f32, tag="sim")
            nc.tensor.matmul(
                po[0:1, 0, 0:8], lhsT=sb_ones[:, :], rhs=sumt[:, :], start=True, stop=True
            )
            so = fpool.tile([1, 8], f32, tag="so")
            nc.scalar.copy(so[:, :], po[0:1, 0, 0:8])
            sw = fpool.tile([1, 8], f32, tag="sw")
            nc.vector.tensor_tensor(
                out=sw[:, :], in0=so[:, :], in1=sb_wvec[:, :], op=mybir.AluOpType.mult
            )
            res = fpool.tile([1, 1], f32, tag="res")
            nc.vector.tensor_reduce(
                out=res[:, :],
                in_=sw[:, :],
                axis=mybir.AxisListType.X,
                op=mybir.AluOpType.add,
            )
            nc.sync.dma_start(d_out[:, :], res[:, :])

    nc.compile()
    return nc


def _prepare_inputs(inputs):
    v_main = np.asarray(inputs["v_main"], np.float32)
    v_aug = np.asarray(inputs["v_aug"], np.float32)
    t_pos = np.asarray(inputs["t_pos"], np.float32)
    t_neg = np.asarray(inputs["t_neg"], np.float32)
    m_pos = np.asarray(inputs["m_pos"]).astype(bool)
    m_neg = np.asarray(inputs["m_neg"]).astype(bool)
    ls = float(np.asarray(inputs["logit_scale"], np.float32))
    s = float(np.exp(np.clip(ls, 0.0, CLAMP_MAX)))

    valid_pos = np.maximum(m_pos.sum(1), 1).astype(np.float32)
    valid_neg = np.maximum(m_neg.sum(1), 1).astype(np.float32)
    jj, llp = np.nonzero(m_pos)
    Kpos = len(jj)
    pos_tok = t_pos[jj, llp, :] / valid_pos[jj][:, None]
    nii, nll = np.nonzero(m_neg)
    maxKn = max(int(np.sum((nii // NL) == c)) for c in range(NCORES)) if len(nii) else 0
    Tp = 128 * int(math.ceil((Kpos + maxKn) / 128)) if (Kpos + maxKn) else 128
    Tb = Tp // 128

    maskN = np.zeros((128, 12), np.float32)
    for il in range(12):
        maskN[96 + il, il] = 1.0
    wvec = (np.array([[1, 1, 1, 1, -2, -2, 1, 1]], np.float32) / 192.0)

    in_maps = []
    for c in range(NCORES):
        tok = np.zeros((Tp, D), np.float32)
        seg = np.zeros((Tp, 108), np.float32)
        tok[:Kpos] = pos_tok
        seg[np.arange(Kpos), jj] = 1.0
        sel = (nii // NL) == c
        ii, lln = nii[sel], nll[sel]
        kneg = len(ii)
        tok[Kpos : Kpos + kneg] = t_neg[ii, lln, :] / valid_neg[ii][:, None]
        seg[Kpos + np.arange(kneg), 96 + ii % NL] = 1.0

        rows = slice(c * NL, (c + 1) * NL)
        vv = np.stack([v_main[rows], v_aug[rows]])  # [2,12,196,128]
        in_maps.append(
            {
                "vT": np.ascontiguousarray(np.transpose(vv, (3, 0, 1, 2))).astype(
                    np.float16
                ),
                "tokT": np.ascontiguousarray(tok.T).astype(np.float16),
                "seg": np.ascontiguousarray(
                    np.transpose(seg.reshape(Tb, 128, 108), (1, 0, 2))
                ).astype(np.float16),
                "maskN": maskN,
                "wvec": wvec,
            }
        )
    return in_maps, Tp, s


def kernel(_trace=False, **inputs):
    from concourse.bass_utils import run_bass_kernel_spmd

    in_maps, Tp, s = _prepare_inputs(inputs)

    key = (Tp, round(s, 9))
    nc = _CACHE.get(key)
    if nc is None:
        nc = _build_program(Tp, s)
        _CACHE[key] = nc

    br = None
    for attempt in range(3):
        try:
            br = run_bass_kernel_spmd(
                nc, in_maps, core_ids=list(range(NCORES)), trace=_trace
            )
            break
        except ModuleNotFoundError:
            # no axon NTFF hook in this container -> run untraced
            _trace = False
        except Exception:
            # transient NRT_EXEC_UNIT_UNRECOVERABLE on the axon terminal has
            # been observed between back-to-back loads; a retry recovers it
            if attempt == 2:
                raise
            import time as _time

            _time.sleep(5.0)
    assert br is not None
    if _trace and br.exec_time_ns is not None:
        kernel.last_exec_time_ns = br.exec_time_ns
    loss = br.results[0]["loss"]
    return np.asarray(loss, np.float32).reshape(())


kernel.last_exec_time_ns = None
